# revision 19
# baseline (speedup 1.0000x reference)
"""Trainium2 Bass kernel for the DualEncoderUNetPP GNN-message-passing head.

Math (per pixel, C=16 classes, D=128 hidden):
  P    = softmax(L)                      (over classes)
  z    = F@L + Mt@P + c0                 F = gnn_w0@feat_w [128,16], Mt = (adj@E)^T
  H0   = relu(z)
  out  = L + Vg@H0 + M2g@P + c1g         (all layer-1 / out-proj params folded,
                                          gate pre-multiplied)

Folding tricks used on-device:
  - c0 is folded into the P-side stage-1 weights (P sums to exactly 1).
  - c1g and M2g are folded into one stage-2 matmul (again via sum(P)=1).
  - the +L residual is a stage-2 matmul against an identity weight image, so
    the final PSUM->SBUF evacuation is a pure copy.

Sharding: data-parallel over 8 cores; core i handles batch b=i//2, pixel half
i%2 of the flattened 512x512 image (131072 px/core).

Device layout: pixels processed in 8 "supers" of 16384 px. A super is a
[128, 2048] tile: partition 16q+c holds class c of chunk q (8 chunks of
2048 px). L is read once (fp32->bf16 cast DMA); the output is written as
bf16 and widened to fp32 on the host.

Per super:
  exp (scalar) -> per-chunk sums (PE, col-tiled wsum) -> recip (DVE) ->
  broadcast recips (SBUF->SBUF DMA, stride-0 replication) -> P=E*r (gpsimd)
  stage1 (PE row-tiled K=32): z[128,1024] per chunk-pair -> relu evac
  (scalar/DVE alternating, bf16) -> stage2 (PE col-tiled M=32): Vg@H0 +
  M2g~@P + I@L -> final copy (scalar/DVE) -> store DMA.
Stage2 of super s is emitted after stage1 of super s+1 (software pipeline)
so the PE never waits on fresh relu evacuations.
"""
import numpy as np
import ml_dtypes
from contextlib import ExitStack

import concourse.bass as bass
import concourse.bacc as bacc
import concourse.tile as tile
import concourse.mybir as mybir
from concourse.bass_utils import run_bass_kernel_spmd

FP32 = mybir.dt.float32
BF16 = mybir.dt.bfloat16
Act = mybir.ActivationFunctionType
Alu = mybir.AluOpType

B, C, H, W = 4, 16, 512, 512
HWIMG = H * W                  # 262144 pixels per image
N_CORES = 8
HWC = B * HWIMG // N_CORES     # 131072 pixels per core
SUP = 16384                    # pixels per super-block
N_SUP = HWC // SUP             # 8
GPS = 4                        # 512-px groups per super (per chunk)

_cached = {}
_last_results = None           # stashed BassKernelResults for test harness

WEIGHT_SPECS = [
    ("wA0", BF16, [128, 128]), ("wA1", BF16, [128, 128]),
    ("wB0", BF16, [128, 128]), ("wB1", BF16, [128, 128]),
    ("wsum", BF16, [128, 8]),
    ("wVe", BF16, [128, 128]), ("wVo", BF16, [128, 128]),
    ("wI", BF16, [128, 128]), ("wM2", BF16, [128, 128]),
]


def _host_constants(inp):
    """Fold the tiny parameter tensors into the kernel's weight images."""
    f32 = lambda k: np.asarray(inp[k], np.float32)
    E = f32("semantic_embeddings")
    relu = lambda x: np.maximum(x, 0)
    e1 = relu(E @ f32("adj_w1").T + f32("adj_b1"))
    e2 = relu(E @ f32("adj_w2").T + f32("adj_b2"))
    adj = 1.0 / (1.0 + np.exp(-(e1 @ e2.T))) + np.eye(C, dtype=np.float32)
    adj = adj / adj.sum(1, keepdims=True)
    gate = float(np.asarray(inp["gate"]))
    M = adj @ E                                             # [C,D]
    F = f32("gnn_w0") @ f32("feat_w")                       # [D,C]
    c0 = f32("gnn_w0") @ f32("feat_b") + f32("gnn_b0")      # [D]
    V = f32("out_w") @ f32("gnn_w1")                        # [C,D]
    M2 = f32("out_w") @ M.T                                 # [C,C]
    c1 = f32("out_w") @ f32("gnn_b1") + f32("out_b")        # [C]
    Vg, M2g, c1g = gate * V, gate * M2, gate * c1
    Mt = M + c0[None, :]                                    # c0 fold: sum(P)=1

    bf = lambda x: np.ascontiguousarray(x, dtype=np.float32).astype(ml_dtypes.bfloat16)
    cst = {}
    for o in range(2):
        wA = np.zeros((128, 128), np.float32)
        wB = np.zeros((128, 128), np.float32)
        for u in range(4):
            wA[32 * u + 16 * o:32 * u + 16 * o + 16, :] = F.T    # [c,d]
            wB[32 * u + 16 * o:32 * u + 16 * o + 16, :] = Mt     # [c,d]
        cst[f"wA{o}"] = bf(wA)
        cst[f"wB{o}"] = bf(wB)
    wsum = np.zeros((128, 8), np.float32)
    for q in range(8):
        wsum[16 * q:16 * q + 16, q] = 1.0
    cst["wsum"] = bf(wsum)
    wVe = np.zeros((128, 128), np.float32)
    wVo = np.zeros((128, 128), np.float32)
    for u in range(4):
        wVe[:, 32 * u:32 * u + 16] = Vg.T                        # [h, d]
        wVo[:, 32 * u + 16:32 * u + 32] = Vg.T
    cst["wVe"] = bf(wVe)
    cst["wVo"] = bf(wVo)
    cst["wI"] = bf(np.eye(128, dtype=np.float32))
    wM2 = np.zeros((128, 128), np.float32)
    for q in range(8):
        wM2[16 * q:16 * q + 16, 16 * q:16 * q + 16] = M2g.T + c1g[None, :]
    cst["wM2"] = bf(wM2)
    return cst


def _declare_io(nc):
    d_L = nc.dram_tensor("Lhw", [C, HWC], FP32, kind="ExternalInput")
    dw = {}
    for name, dt_, shape in WEIGHT_SPECS:
        dw[name] = nc.dram_tensor(name, shape, dt_, kind="ExternalInput")
    d_out = nc.dram_tensor("out", [C, HWC], BF16, kind="ExternalOutput")
    return d_L, dw, d_out


def _load_consts(nc, tc, const, dw):
    t = {}
    for name, dt_, shape in WEIGHT_SPECS:
        tt = const.tile(shape, dt_, tag=name)
        nc.sync.dma_start(out=tt, in_=dw[name][:])
        t[name] = tt
    return t


class _SuperState:
    """Tiles produced by head/stage1 of a super, consumed downstream."""
    __slots__ = ("tL", "tP", "h0", "base")


def _head_body(nc, t, d_L, pools, base, parts):
    """Load + full softmax chain: produces tL (bf16 logits) and tP (probs)."""
    DMA = "dma" in parts; PE = "pe" in parts; EW = "ew" in parts
    sb, psZ, psS, psO, h0p = pools
    st = _SuperState()
    st.base = base

    # ---- load L (fp32 -> bf16 cast DMA) ----
    tL = sb.tile([128, 2048], BF16, tag="tL")
    st.tL = tL
    if DMA:
        src = bass.AP(d_L[:].tensor, base, [[2048, 8], [HWC, 16], [1, 2048]])
        nc.gpsimd.dma_start(out=tL, in_=src)
    else:
        nc.vector.memset(tL[:, 0:1], 0.0)

    # ---- softmax pieces ----
    tE = sb.tile([128, 2048], BF16, tag="tE")
    if EW:
        nc.scalar.activation(tE, tL, Act.Exp)
    else:
        nc.vector.memset(tE[:, 0:1], 0.0)
    p_s = psS.tile([128, 512], FP32, tag="ps")
    if PE:
        for g in range(GPS):
            nc.tensor.matmul(p_s[32 * g:32 * g + 8, :], t["wsum"][:],
                             tE[:, 512 * g:512 * (g + 1)],
                             start=True, stop=True, tile_position=(0, 32 * g))
    else:
        nc.vector.memset(p_s[:, 0:1], 1.0)
    t_rs = sb.tile([104, 512], FP32, tag="rs")
    t_rsb = sb.tile([104, 512], BF16, tag="rsb")
    if EW:
        nc.vector.reciprocal_approx_fast(out=t_rs, in_=p_s[0:104, :])
        nc.gpsimd.tensor_copy(t_rsb, t_rs)
    else:
        nc.vector.memset(t_rs[:, 0:1], 0.0)
        nc.vector.memset(t_rsb[:, 0:1], 0.0)

    # ---- broadcast recips to [128, 2048] (SBUF->SBUF DMA, stride-0) ----
    t_bc = sb.tile([128, 2048], BF16, tag="bc")
    if DMA and EW:
        for g in range(GPS):
            src = bass.AP(t_rsb[:].tensor, 512 * 32 * g,
                          [[512, 8], [0, 16], [1, 512]])
            nc.sync.dma_start(out=t_bc[:, 512 * g:512 * (g + 1)], in_=src)
    else:
        nc.vector.memset(t_bc[:, 0:1], 0.0)

    # ---- P = E * r (gpsimd, SBUF only) ----
    tP = sb.tile([128, 2048], BF16, tag="tP")
    st.tP = tP
    if EW:
        nc.gpsimd.tensor_mul(tP, tE, t_bc)
    else:
        nc.vector.memset(tP[:, 0:1], 0.0)
    return st


def _stage1_body(nc, t, pools, st, parts):
    """z = F@L + Mt@P per chunk (PE row-tiled); relu evac to bf16 H0."""
    PE = "pe" in parts; EW = "ew" in parts
    sb, psZ, psS, psO, h0p = pools
    tL, tP = st.tL, st.tP
    st.h0 = {}
    for g in range(GPS):
        cols = slice(512 * g, 512 * (g + 1))
        for u in range(4):
            z = psZ.tile([128, 1024], FP32, tag="z")
            if PE:
                for o in range(2):          # chunk q = 2u+o -> cols 512*o
                    zc = z[:, 512 * o:512 * (o + 1)]
                    nc.tensor.matmul(zc, t[f"wA{o}"][32 * u:32 * u + 32, :],
                                     tL[32 * u:32 * u + 32, cols],
                                     start=True, stop=False,
                                     tile_position=(32 * u, 0))
                    nc.tensor.matmul(zc, t[f"wB{o}"][32 * u:32 * u + 32, :],
                                     tP[32 * u:32 * u + 32, cols],
                                     start=False, stop=True,
                                     tile_position=(32 * u, 0))
            else:
                nc.vector.memset(z[:, 0:1], 0.0)
            h0 = h0p.tile([128, 1024], BF16, tag=f"h0{u}")
            if EW:
                if (g * 4 + u) % 2 == 0:
                    nc.scalar.activation(h0, z, Act.Relu)
                else:
                    nc.vector.tensor_scalar_max(h0, z, 0.0)
            else:
                nc.vector.memset(h0[:, 0:1], 0.0)
            st.h0[(u, g)] = h0
    return st


def _stage2_body(nc, t, d_out, pools, st, parts):
    DMA = "dma" in parts; PE = "pe" in parts; EW = "ew" in parts
    sb, psZ, psS, psO, h0p = pools
    t_out = sb.tile([128, 2048], BF16, tag="out")
    for g in range(GPS):
        cols = slice(512 * g, 512 * (g + 1))
        o2 = psO.tile([128, 512], FP32, tag="o2")
        if PE:
            for u in range(4):
                osl = o2[32 * u:32 * u + 32, :]
                h0 = st.h0[(u, g)]
                nc.tensor.matmul(osl, t["wVe"][:, 32 * u:32 * u + 32],
                                 h0[:, 0:512], start=True, stop=False,
                                 tile_position=(0, 32 * u))
                nc.tensor.matmul(osl, t["wVo"][:, 32 * u:32 * u + 32],
                                 h0[:, 512:1024], start=False, stop=False,
                                 tile_position=(0, 32 * u))
                nc.tensor.matmul(osl, t["wI"][:, 32 * u:32 * u + 32],
                                 st.tL[:, cols], start=False, stop=False,
                                 tile_position=(0, 32 * u))
                nc.tensor.matmul(osl, t["wM2"][:, 32 * u:32 * u + 32],
                                 st.tP[:, cols], start=False, stop=True,
                                 tile_position=(0, 32 * u))
        else:
            nc.vector.memset(o2[:, 0:1], 0.0)
        if EW:
            if g % 2 == 0:
                nc.vector.tensor_copy(t_out[:, cols], o2)
            else:
                nc.scalar.copy(t_out[:, cols], o2)
        else:
            nc.vector.memset(t_out[:, 0:1], 0.0)
    if DMA:
        dst = bass.AP(d_out[:].tensor, st.base,
                      [[2048, 8], [HWC, 16], [1, 2048]])
        nc.sync.dma_start(out=dst, in_=t_out)


def _emit_pass(nc, t, d_L, d_out, pools, parts, bases):
    """Pipeline (v2 order): head(i)+stage1(i), then stage2(i-1)."""
    sts = []
    n = len(bases)
    for i in range(n):
        sts.append(_head_body(nc, t, d_L, pools, bases[i], parts))
        _stage1_body(nc, t, pools, sts[i], parts)
        if i >= 1:
            _stage2_body(nc, t, d_out, pools, sts[i - 1], parts)
    _stage2_body(nc, t, d_out, pools, sts[n - 1], parts)


def _make_pools(nc, tc, ctx):
    const = ctx.enter_context(tc.tile_pool(name="const", bufs=1))
    sb = ctx.enter_context(tc.tile_pool(name="sb", bufs=3))
    psZ = ctx.enter_context(tc.tile_pool(name="psZ", bufs=2, space="PSUM"))
    psS = ctx.enter_context(tc.tile_pool(name="psS", bufs=1, space="PSUM"))
    psO = ctx.enter_context(tc.tile_pool(name="psO", bufs=2, space="PSUM"))
    h0p = ctx.enter_context(tc.tile_pool(name="h0p", bufs=8))
    return const, (sb, psZ, psS, psO, h0p)


def _build_program(reps=1):
    """Build the SPMD single-core program (identical on all 8 cores)."""
    nc = bacc.Bacc("TRN2", target_bir_lowering=False, debug=False)
    d_L, dw, d_out = _declare_io(nc)
    with ExitStack() as ctx:
        tc = ctx.enter_context(tile.TileContext(nc))
        const, pools = _make_pools(nc, tc, ctx)
        t = _load_consts(nc, tc, const, dw)
        bases = [(s % N_SUP) * SUP for s in range(N_SUP * reps)]
        _emit_pass(nc, t, d_L, d_out, pools, ("dma", "pe", "ew"), bases)
    nc.compile()
    return nc


def _build_loop_program(iters, parts=("dma", "pe", "ew"), bodyk=1):
    """bodyk super-bodies inside a dynamic For_i loop (timing harness)."""
    nc = bacc.Bacc("TRN2", target_bir_lowering=False, debug=False)
    d_L, dw, d_out = _declare_io(nc)
    with ExitStack() as ctx:
        tc = ctx.enter_context(tile.TileContext(nc))
        const, pools = _make_pools(nc, tc, ctx)
        t = _load_consts(nc, tc, const, dw)
        sb, psZ, psS, psO, h0p = pools
        # pre-zero rotating buffers so partial-engine benches read finite data
        if "pe" not in parts:
            for _ in range(2):
                nc.vector.memset(psZ.tile([128, 1024], FP32, tag="z"), 0.0)
            nc.vector.memset(psS.tile([128, 512], FP32, tag="ps"), 1.0)
            for _ in range(2):
                nc.vector.memset(psO.tile([128, 512], FP32, tag="o2"), 0.0)
        if "dma" not in parts or "ew" not in parts:
            for _ in range(3):
                nc.vector.memset(sb.tile([128, 2048], BF16, tag="tL"), 0.0)
                nc.vector.memset(sb.tile([128, 2048], BF16, tag="tE"), 1.0)
                nc.vector.memset(sb.tile([128, 2048], BF16, tag="tP"), 0.0)
                nc.vector.memset(sb.tile([128, 2048], BF16, tag="bc"), 1.0)
                nc.vector.memset(sb.tile([104, 512], BF16, tag="rsb"), 1.0)
            for u in range(4):
                for _ in range(8):
                    nc.vector.memset(h0p.tile([128, 1024], BF16, tag=f"h0{u}"), 0.0)
        with tc.For_i(0, iters, 1):
            bases = [(k % N_SUP) * SUP for k in range(bodyk)]
            _emit_pass(nc, t, d_L, d_out, pools, parts, bases)
    nc.compile()
    return nc


def kernel(**inputs):
    global _last_results
    if "nc" not in _cached:
        _cached["nc"] = _build_program()
    nc = _cached["nc"]
    cst = _host_constants(inputs)
    L = np.asarray(inputs["class_logits"], np.float32).reshape(B, C, HWIMG)
    in_maps = []
    for i in range(N_CORES):
        b, half = i // 2, i % 2
        slab = np.ascontiguousarray(L[b][:, half * HWC:(half + 1) * HWC])
        m = {"Lhw": slab}
        m.update(cst)
        in_maps.append(m)
    res = run_bass_kernel_spmd(nc, in_maps, list(range(N_CORES)),
                               trace=bool(_cached.get("trace", False)))
    _last_results = res
    out = np.empty((B, C, HWIMG), np.float32)
    for i in range(N_CORES):
        b, half = i // 2, i % 2
        out[b][:, half * HWC:(half + 1) * HWC] = \
            np.asarray(res.results[i]["out"]).astype(np.float32)
    return out.reshape(B, C, H, W)


# revision 20
# speedup vs baseline: 1.4717x; 1.4717x over previous
"""Trainium2 Bass kernel for the DualEncoderUNetPP GNN-message-passing head.

Math (per pixel, C=16 classes, D=128 hidden):
  P    = softmax(L)                      (over classes)
  z    = F@L + Mt@P + c0                 F = gnn_w0@feat_w [128,16], Mt = (adj@E)^T
  H0   = relu(z)
  out  = L + Vg@H0 + M2g@P + c1g         (all layer-1 / out-proj params folded,
                                          gate pre-multiplied)

Folding tricks used on-device:
  - c0 is folded into the P-side stage-1 weights (P sums to exactly 1).
  - c1g and M2g are folded into one stage-2 matmul (again via sum(P)=1).
  - the +L residual is a stage-2 matmul against an identity weight image, so
    the final PSUM->SBUF evacuation is a pure copy.

Sharding: data-parallel over 8 cores; core i handles batch b=i//2, pixel half
i%2 of the flattened 512x512 image (131072 px/core).

Device layout: pixels processed in 8 "supers" of 16384 px. A super is a
[128, 2048] tile: partition 16q+c holds class c of chunk q (8 chunks of
2048 px). L is read once (fp32->bf16 cast DMA); the output is written as
bf16 and widened to fp32 on the host.

Per super:
  exp (scalar) -> per-chunk sums (PE, col-tiled wsum) -> recip (DVE) ->
  broadcast recips (SBUF->SBUF DMA, stride-0 replication) -> P=E*r (gpsimd)
  stage1 (PE row-tiled K=32): z[128,1024] per chunk-pair -> relu evac
  (scalar/DVE alternating, bf16) -> stage2 (PE col-tiled M=32): Vg@H0 +
  M2g~@P + I@L -> final copy (scalar/DVE) -> store DMA.
Stage2 of super s is emitted after stage1 of super s+1 (software pipeline)
so the PE never waits on fresh relu evacuations.
"""
import numpy as np
import ml_dtypes
from contextlib import ExitStack

import concourse.bass as bass
import concourse.bacc as bacc
import concourse.tile as tile
import concourse.mybir as mybir
from concourse.bass_utils import run_bass_kernel_spmd

FP32 = mybir.dt.float32
BF16 = mybir.dt.bfloat16
Act = mybir.ActivationFunctionType
Alu = mybir.AluOpType

B, C, H, W = 4, 16, 512, 512
HWIMG = H * W                  # 262144 pixels per image
N_CORES = 8
HWC = B * HWIMG // N_CORES     # 131072 pixels per core
SUP = 16384                    # pixels per super-block
N_SUP = HWC // SUP             # 8
GPS = 4                        # 512-px groups per super (per chunk)

_cached = {}
_last_results = None           # stashed BassKernelResults for test harness

WEIGHT_SPECS = [
    ("wA0", BF16, [128, 128]), ("wA1", BF16, [128, 128]),
    ("wB0", BF16, [128, 128]), ("wB1", BF16, [128, 128]),
    ("wsum", BF16, [128, 8]), ("wbc", BF16, [128, 128]),
    ("wVe", BF16, [128, 128]), ("wVo", BF16, [128, 128]),
    ("wI", BF16, [128, 128]), ("wM2", BF16, [128, 128]),
]


def _host_constants(inp):
    """Fold the tiny parameter tensors into the kernel's weight images."""
    f32 = lambda k: np.asarray(inp[k], np.float32)
    E = f32("semantic_embeddings")
    relu = lambda x: np.maximum(x, 0)
    e1 = relu(E @ f32("adj_w1").T + f32("adj_b1"))
    e2 = relu(E @ f32("adj_w2").T + f32("adj_b2"))
    adj = 1.0 / (1.0 + np.exp(-(e1 @ e2.T))) + np.eye(C, dtype=np.float32)
    adj = adj / adj.sum(1, keepdims=True)
    gate = float(np.asarray(inp["gate"]))
    M = adj @ E                                             # [C,D]
    F = f32("gnn_w0") @ f32("feat_w")                       # [D,C]
    c0 = f32("gnn_w0") @ f32("feat_b") + f32("gnn_b0")      # [D]
    V = f32("out_w") @ f32("gnn_w1")                        # [C,D]
    M2 = f32("out_w") @ M.T                                 # [C,C]
    c1 = f32("out_w") @ f32("gnn_b1") + f32("out_b")        # [C]
    Vg, M2g, c1g = gate * V, gate * M2, gate * c1
    Mt = M + c0[None, :]                                    # c0 fold: sum(P)=1

    bf = lambda x: np.ascontiguousarray(x, dtype=np.float32).astype(ml_dtypes.bfloat16)
    cst = {}
    for o in range(2):
        wA = np.zeros((128, 128), np.float32)
        wB = np.zeros((128, 128), np.float32)
        for u in range(4):
            wA[32 * u + 16 * o:32 * u + 16 * o + 16, :] = F.T    # [c,d]
            wB[32 * u + 16 * o:32 * u + 16 * o + 16, :] = Mt     # [c,d]
        cst[f"wA{o}"] = bf(wA)
        cst[f"wB{o}"] = bf(wB)
    wsum = np.zeros((128, 8), np.float32)
    for q in range(8):
        wsum[16 * q:16 * q + 16, q] = 1.0
    cst["wsum"] = bf(wsum)
    wbc = np.zeros((128, 128), np.float32)
    for g in range(4):
        for p in range(128):
            wbc[32 * g + p // 16, p] = 1.0
    cst["wbc"] = bf(wbc)
    wVe = np.zeros((128, 128), np.float32)
    wVo = np.zeros((128, 128), np.float32)
    for u in range(4):
        wVe[:, 32 * u:32 * u + 16] = Vg.T                        # [h, d]
        wVo[:, 32 * u + 16:32 * u + 32] = Vg.T
    cst["wVe"] = bf(wVe)
    cst["wVo"] = bf(wVo)
    cst["wI"] = bf(np.eye(128, dtype=np.float32))
    wM2 = np.zeros((128, 128), np.float32)
    for q in range(8):
        wM2[16 * q:16 * q + 16, 16 * q:16 * q + 16] = M2g.T + c1g[None, :]
    cst["wM2"] = bf(wM2)
    return cst


def _declare_io(nc):
    d_L = nc.dram_tensor("Lhw", [C, HWC], FP32, kind="ExternalInput")
    dw = {}
    for name, dt_, shape in WEIGHT_SPECS:
        dw[name] = nc.dram_tensor(name, shape, dt_, kind="ExternalInput")
    d_out = nc.dram_tensor("out", [C, HWC], BF16, kind="ExternalOutput")
    return d_L, dw, d_out


def _load_consts(nc, tc, const, dw):
    t = {}
    for name, dt_, shape in WEIGHT_SPECS:
        tt = const.tile(shape, dt_, tag=name)
        nc.sync.dma_start(out=tt, in_=dw[name][:])
        t[name] = tt
    return t


class _SuperState:
    """Tiles produced by head/stage1 of a super, consumed downstream."""
    __slots__ = ("tL", "tP", "h0", "base")


def _head_body(nc, t, d_L, pools, base, parts):
    """Load + full softmax chain: produces tL (bf16 logits) and tP (probs)."""
    DMA = "dma" in parts; PE = "pe" in parts; EW = "ew" in parts
    sb, psZ, psS, psO, h0p = pools
    st = _SuperState()
    st.base = base

    # ---- load L (fp32 -> bf16 cast DMA) ----
    tL = sb.tile([128, 2048], BF16, tag="tL")
    st.tL = tL
    if DMA:
        src = bass.AP(d_L[:].tensor, base, [[2048, 8], [HWC, 16], [1, 2048]])
        nc.gpsimd.dma_start(out=tL, in_=src)
    else:
        nc.vector.memset(tL[:, 0:1], 0.0)

    # ---- softmax pieces ----
    tE = sb.tile([128, 2048], BF16, tag="tE")
    if EW:
        nc.scalar.activation(tE, tL, Act.Exp)
    else:
        nc.vector.memset(tE[:, 0:1], 0.0)
    p_s = psS.tile([128, 512], FP32, tag="ps")
    if PE:
        for g in range(GPS):
            nc.tensor.matmul(p_s[32 * g:32 * g + 8, :], t["wsum"][:],
                             tE[:, 512 * g:512 * (g + 1)],
                             start=True, stop=True, tile_position=(0, 32 * g))
    else:
        nc.vector.memset(p_s[:, 0:1], 1.0)
    t_rs = sb.tile([104, 512], FP32, tag="rs")
    t_rsb = sb.tile([104, 512], BF16, tag="rsb")
    if EW:
        nc.vector.reciprocal_approx_fast(out=t_rs, in_=p_s[0:104, :])
        nc.vector.tensor_copy(t_rsb, t_rs)
    else:
        nc.vector.memset(t_rs[:, 0:1], 0.0)
        nc.vector.memset(t_rsb[:, 0:1], 0.0)

    # ---- P = E * (1/S) via wbc-matmul broadcast + per-group DVE muls ----
    tP = sb.tile([128, 2048], BF16, tag="tP")
    st.tP = tP
    for g in range(GPS):
        p_bc = psS.tile([128, 512], FP32, tag="ps")
        if not PE:
            nc.vector.memset(p_bc[:, 0:1], 1.0)
        else:
            nc.tensor.matmul(p_bc, t["wbc"][32 * g:32 * g + 8, :],
                             t_rsb[32 * g:32 * g + 8, :],
                             start=True, stop=True, tile_position=(32 * g, 0))
        if EW:
            nc.vector.tensor_mul(tP[:, 512 * g:512 * (g + 1)],
                                 tE[:, 512 * g:512 * (g + 1)], p_bc)
        else:
            nc.vector.memset(tP[:, 0:1], 0.0)
    return st


def _stage1_body(nc, t, pools, st, parts):
    """z = F@L + Mt@P per chunk (PE row-tiled); relu evac to bf16 H0."""
    PE = "pe" in parts; EW = "ew" in parts
    sb, psZ, psS, psO, h0p = pools
    tL, tP = st.tL, st.tP
    st.h0 = {}
    for g in range(GPS):
        cols = slice(512 * g, 512 * (g + 1))
        for u in range(4):
            z = psZ.tile([128, 1024], FP32, tag="z")
            if PE:
                for o in range(2):          # chunk q = 2u+o -> cols 512*o
                    zc = z[:, 512 * o:512 * (o + 1)]
                    nc.tensor.matmul(zc, t[f"wA{o}"][32 * u:32 * u + 32, :],
                                     tL[32 * u:32 * u + 32, cols],
                                     start=True, stop=False,
                                     tile_position=(32 * u, 0))
                    nc.tensor.matmul(zc, t[f"wB{o}"][32 * u:32 * u + 32, :],
                                     tP[32 * u:32 * u + 32, cols],
                                     start=False, stop=True,
                                     tile_position=(32 * u, 0))
            else:
                nc.vector.memset(z[:, 0:1], 0.0)
            h0 = h0p.tile([128, 1024], BF16, tag=f"h0{u}")
            if EW:
                if (g * 4 + u) % 2 == 0:
                    nc.scalar.activation(h0, z, Act.Relu)
                else:
                    nc.vector.tensor_scalar_max(h0, z, 0.0)
            else:
                nc.vector.memset(h0[:, 0:1], 0.0)
            st.h0[(u, g)] = h0
    return st


def _stage2_body(nc, t, d_out, pools, st, parts):
    DMA = "dma" in parts; PE = "pe" in parts; EW = "ew" in parts
    sb, psZ, psS, psO, h0p = pools
    t_out = sb.tile([128, 2048], BF16, tag="out")
    for g in range(GPS):
        cols = slice(512 * g, 512 * (g + 1))
        o2 = psO.tile([128, 512], FP32, tag="o2")
        if PE:
            for u in range(4):
                osl = o2[32 * u:32 * u + 32, :]
                h0 = st.h0[(u, g)]
                nc.tensor.matmul(osl, t["wVe"][:, 32 * u:32 * u + 32],
                                 h0[:, 0:512], start=True, stop=False,
                                 tile_position=(0, 32 * u))
                nc.tensor.matmul(osl, t["wVo"][:, 32 * u:32 * u + 32],
                                 h0[:, 512:1024], start=False, stop=False,
                                 tile_position=(0, 32 * u))
                nc.tensor.matmul(osl, t["wI"][:, 32 * u:32 * u + 32],
                                 st.tL[:, cols], start=False, stop=False,
                                 tile_position=(0, 32 * u))
                nc.tensor.matmul(osl, t["wM2"][:, 32 * u:32 * u + 32],
                                 st.tP[:, cols], start=False, stop=True,
                                 tile_position=(0, 32 * u))
        else:
            nc.vector.memset(o2[:, 0:1], 0.0)
        if EW:
            if g % 2 == 0:
                nc.vector.tensor_copy(t_out[:, cols], o2)
            else:
                nc.scalar.copy(t_out[:, cols], o2)
        else:
            nc.vector.memset(t_out[:, 0:1], 0.0)
    if DMA:
        dst = bass.AP(d_out[:].tensor, st.base,
                      [[2048, 8], [HWC, 16], [1, 2048]])
        nc.sync.dma_start(out=dst, in_=t_out)


def _emit_pass(nc, t, d_L, d_out, pools, parts, bases):
    """Pipeline: head one super ahead; stage2 one behind stage1."""
    sts = []
    n = len(bases)
    sts.append(_head_body(nc, t, d_L, pools, bases[0], parts))
    for i in range(n + 1):
        if i + 1 < n:
            sts.append(_head_body(nc, t, d_L, pools, bases[i + 1], parts))
        if i < n:
            _stage1_body(nc, t, pools, sts[i], parts)
        if i >= 1:
            _stage2_body(nc, t, d_out, pools, sts[i - 1], parts)


def _make_pools(nc, tc, ctx):
    const = ctx.enter_context(tc.tile_pool(name="const", bufs=1))
    sb = ctx.enter_context(tc.tile_pool(name="sb", bufs=4))
    psZ = ctx.enter_context(tc.tile_pool(name="psZ", bufs=3, space="PSUM"))
    psS = ctx.enter_context(tc.tile_pool(name="psS", bufs=1, space="PSUM"))
    psO = ctx.enter_context(tc.tile_pool(name="psO", bufs=1, space="PSUM"))
    h0p = ctx.enter_context(tc.tile_pool(name="h0p", bufs=8))
    return const, (sb, psZ, psS, psO, h0p)


def _build_program(reps=1):
    """Build the SPMD single-core program (identical on all 8 cores)."""
    nc = bacc.Bacc("TRN2", target_bir_lowering=False, debug=False)
    d_L, dw, d_out = _declare_io(nc)
    with ExitStack() as ctx:
        tc = ctx.enter_context(tile.TileContext(nc))
        const, pools = _make_pools(nc, tc, ctx)
        t = _load_consts(nc, tc, const, dw)
        bases = [(s % N_SUP) * SUP for s in range(N_SUP * reps)]
        _emit_pass(nc, t, d_L, d_out, pools, ("dma", "pe", "ew"), bases)
    nc.compile()
    return nc


def _build_loop_program(iters, parts=("dma", "pe", "ew"), bodyk=1):
    """bodyk super-bodies inside a dynamic For_i loop (timing harness)."""
    nc = bacc.Bacc("TRN2", target_bir_lowering=False, debug=False)
    d_L, dw, d_out = _declare_io(nc)
    with ExitStack() as ctx:
        tc = ctx.enter_context(tile.TileContext(nc))
        const, pools = _make_pools(nc, tc, ctx)
        t = _load_consts(nc, tc, const, dw)
        sb, psZ, psS, psO, h0p = pools
        # pre-zero rotating buffers so partial-engine benches read finite data
        if "pe" not in parts:
            for _ in range(3):
                nc.vector.memset(psZ.tile([128, 1024], FP32, tag="z"), 0.0)
            nc.vector.memset(psS.tile([128, 512], FP32, tag="ps"), 1.0)
            nc.vector.memset(psO.tile([128, 512], FP32, tag="o2"), 0.0)
        if "dma" not in parts or "ew" not in parts:
            for _ in range(3):
                nc.vector.memset(sb.tile([128, 2048], BF16, tag="tL"), 0.0)
                nc.vector.memset(sb.tile([128, 2048], BF16, tag="tE"), 1.0)
                nc.vector.memset(sb.tile([128, 2048], BF16, tag="tP"), 0.0)
                nc.vector.memset(sb.tile([104, 512], BF16, tag="rsb"), 1.0)
            for u in range(4):
                for _ in range(8):
                    nc.vector.memset(h0p.tile([128, 1024], BF16, tag=f"h0{u}"), 0.0)
        with tc.For_i(0, iters, 1):
            bases = [(k % N_SUP) * SUP for k in range(bodyk)]
            _emit_pass(nc, t, d_L, d_out, pools, parts, bases)
    nc.compile()
    return nc


def kernel(**inputs):
    global _last_results
    if "nc" not in _cached:
        _cached["nc"] = _build_program()
    nc = _cached["nc"]
    cst = _host_constants(inputs)
    L = np.asarray(inputs["class_logits"], np.float32).reshape(B, C, HWIMG)
    in_maps = []
    for i in range(N_CORES):
        b, half = i // 2, i % 2
        slab = np.ascontiguousarray(L[b][:, half * HWC:(half + 1) * HWC])
        m = {"Lhw": slab}
        m.update(cst)
        in_maps.append(m)
    res = run_bass_kernel_spmd(nc, in_maps, list(range(N_CORES)),
                               trace=bool(_cached.get("trace", False)))
    _last_results = res
    out = np.empty((B, C, HWIMG), np.float32)
    for i in range(N_CORES):
        b, half = i // 2, i % 2
        out[b][:, half * HWC:(half + 1) * HWC] = \
            np.asarray(res.results[i]["out"]).astype(np.float32)
    return out.reshape(B, C, H, W)
